# revision 34
# baseline (speedup 1.0000x reference)
"""Trainium2 Bass kernel for windowed multi-head self-attention (Swin/LSA style).

Shapes (hardcoded): x [2048, 50, 256], 8 heads, head_dim 32, window N=50
(49 patch tokens + 1 region token), relative-position bias on the 49x49 block.

Strategy: data-parallel over the 2048 windows across 8 NeuronCores (256
windows/core). Per core, tokens are processed in chunks of 2 windows
(100 tokens), software-pipelined by one chunk so the PE never waits on the
exp activation. All matmuls bf16 on the PE with fp32 PSUM accumulate.

Per chunk (scoresT layout [key, (h,q)], no transposes, no block-diag build):
  scores: 8 row-tiled matmuls (K=32 per head, tile_position row strips) that
    accumulate onto a bias+mask pre-injected by 2 identity matmuls
  exp via one ACT op over both PSUM banks during evac
  AV: 8 col-tiled matmuls (M=32, tile_position col strips) -> dense PSUM
    layout [row 32*(h%4)+d, bank h//4]
  denominators: 8 col-tiled ones-matmuls into spare cols of the same banks
    (each [32,100] block = colsums broadcast over 32 rows, matching layout)
  reciprocal_approx_fast on the denom cols, 2 evac-multiplies -> dense avsb
  proj reversed (outT = Wp^T.T @ avsb): 2 accumulating matmuls per 400-col
    tile, output written transposed to HBM; host does the final transpose.
"""
import os
import sys
import numpy as np
import ml_dtypes

sys.path.insert(0, '/opt/trn_rl_repo')

import concourse.bacc as bacc
import concourse.mybir as mybir
from concourse import tile
from concourse.bass_utils import run_bass_kernel_spmd

BF16 = mybir.dt.bfloat16
F32 = mybir.dt.float32

N_CORES = 8
DIM = 256
H = 8
HD = 32
WN = 50                      # tokens per window
B_ = 2048
BPC = B_ // N_CORES          # windows per core
T = BPC * WN                 # tokens per core = 12800
CT = 100                     # tokens per chunk (2 windows)
NCHUNK = T // CT             # 128 chunks per core
BLK_CH = 32                  # chunks per block
BLK_T = BLK_CH * CT          # 3200 tokens per block
NBLK = NCHUNK // BLK_CH      # 4 blocks

_cache = {}
SAFE_RCP = bool(os.environ.get("K_SAFE_RCP"))
SAFE_SC = True   # grouped bias-inject + skip_group_check hangs on HW
SAFE_EXP = bool(os.environ.get("K_SAFE_EXP"))


def _install_ntff_shim():
    """Register the axon NTFF profile hook (antenv stub lacks axon_hooks)."""
    import types
    if 'antenv.axon_hooks' in sys.modules:
        return
    try:
        import antenv
        from trn_agent_boot.trn_boot import _ntff_profile_via_ctypes
    except ImportError:
        return
    hooks = types.ModuleType("antenv.axon_hooks")
    holder = {}
    hooks.set_axon_ntff_profile_hook = lambda h: holder.__setitem__('h', h)
    hooks.get_axon_ntff_profile_hook = lambda: holder.get('h')
    antenv.axon_hooks = hooks
    sys.modules['antenv.axon_hooks'] = hooks
    hook = _ntff_profile_via_ctypes('/opt/axon/libaxon_pjrt.so')
    if hook is not None:
        hooks.set_axon_ntff_profile_hook(hook)


def _build_program():
    if 'nc' in _cache:
        return _cache['nc']
    nc = bacc.Bacc("TRN2", target_bir_lowering=False, debug=False,
                   num_devices=N_CORES)
    x_d = nc.dram_tensor("x", [T, DIM], BF16, kind="ExternalInput").ap()
    wqk_d = nc.dram_tensor("wqk", [128, 1024], BF16, kind="ExternalInput").ap()
    wv_d = nc.dram_tensor("wv", [128, 512], BF16, kind="ExternalInput").ap()
    wp_d = nc.dram_tensor("wpT", [128, 512], BF16, kind="ExternalInput").ap()
    bias_d = nc.dram_tensor("biasT", [100, 800], BF16, kind="ExternalInput").ap()
    qb_d = nc.dram_tensor("qb", [128, 4], F32, kind="ExternalInput").ap()
    out_d = nc.dram_tensor("out", [128, 2 * T], BF16, kind="ExternalOutput").ap()

    from contextlib import ExitStack
    with tile.TileContext(nc) as tc, ExitStack() as es:
        cpool = es.enter_context(tc.tile_pool(name="consts", bufs=1))
        wqk = cpool.tile([128, 1024], BF16)       # [ct, 4mt x 128] qk weights
        nc.sync.dma_start(out=wqk[:], in_=wqk_d[:])
        wv = cpool.tile([128, 512], BF16)         # [ct, 256] v weights (rhs)
        nc.sync.dma_start(out=wv[:], in_=wv_d[:])
        wpT = cpool.tile([128, 512], BF16)        # [(j,d), (s,t,128)] proj^T
        nc.sync.dma_start(out=wpT[:], in_=wp_d[:])
        expB = cpool.tile([100, 800], BF16)       # exp(bias), 0 at junk
        nc.sync.dma_start(out=expB[:], in_=bias_d[:])
        qb = cpool.tile([128, 4], F32)            # q/k bias per-partition
        nc.sync.dma_start(out=qb[:], in_=qb_d[:])
        ones32 = cpool.tile([100, 32], BF16)      # denominator stationary
        nc.vector.memset(ones32[:], 1.0)

        xt_pool = es.enter_context(tc.tile_pool(name="xt", bufs=2))
        qk_pool = es.enter_context(tc.tile_pool(name="qk", bufs=2))
        v_pool = es.enter_context(tc.tile_pool(name="v", bufs=2))
        a_pool = es.enter_context(tc.tile_pool(name="attn", bufs=2))
        r_pool = es.enter_context(tc.tile_pool(name="rcp", bufs=2))
        av_pool = es.enter_context(tc.tile_pool(name="avsb", bufs=2))
        o_pool = es.enter_context(tc.tile_pool(name="osb", bufs=2))
        ps_qk = es.enter_context(tc.tile_pool(name="ps_qk", bufs=2, space="PSUM"))
        ps_s = es.enter_context(tc.tile_pool(name="ps_s", bufs=1, space="PSUM"))
        ps_av = es.enter_context(tc.tile_pool(name="ps_av", bufs=1, space="PSUM"))
        ps_o = es.enter_context(tc.tile_pool(name="ps_o", bufs=1, space="PSUM"))

        xt = [None, None]   # double-buffered via pool tags

        def load_xt(b):
            t0 = b * BLK_T
            tls = [xt_pool.tile([128, BLK_T], BF16, tag=f"xt{ct}",
                                name=f"xt{ct}_{b}") for ct in range(2)]
            for ct in range(2):
                nc.sync.dma_start(out=tls[ct][:],
                                  in_=x_d[t0:t0 + BLK_T, 128*ct:128*ct+128],
                                  transpose=True)
            return tls

        xt_cur = load_xt(0)
        for b in range(NBLK):
            t0 = b * BLK_T
            xt = xt_cur
            # qT/kT: 4 m-tiles [128, 3200] (q g0, q g1, k g0, k g1),
            # produced in 400-col tiles interleaved with chunk work below
            qk = [qk_pool.tile([128, BLK_T], BF16, tag=f"qk{mt}",
                               name=f"qk{mt}_{b}") for mt in range(4)]

            vchs = {}

            def qk_pairs(i, j):
                # j=0/1: qk m-tile pairs; j=2/3: v pairs — spread across the
                # group so the PE has filler work during every exp window
                ns = 400 * i
                if j < 2:
                    for mt in (2*j, 2*j+1):
                        ps = ps_qk.tile([128, 512], F32, tag="ps_qk")
                        for ct in range(2):
                            nc.tensor.matmul(
                                ps[:, 0:400],
                                wqk[:, ct*512 + mt*128: ct*512 + mt*128+128],
                                xt[ct][:, ns:ns+400],
                                start=(ct == 0), stop=(ct == 1))
                        with nc.allow_low_precision(reason="bf16 qk"):
                            if mt >= 2:
                                nc.vector.tensor_scalar_add(
                                    qk[mt][:, ns:ns+400], ps[:, 0:400],
                                    qb[:, mt:mt+1])
                            else:
                                nc.scalar.activation(
                                    qk[mt][:, ns:ns+400], ps[:, 0:400],
                                    mybir.ActivationFunctionType.Identity,
                                    bias=qb[:, mt:mt+1])
                else:
                    for k in (2*(j-2), 2*(j-2)+1):
                        c0v = ns + 100 * k
                        ps = ps_qk.tile([128, 512], F32, tag="ps_qk")
                        for ct in range(2):
                            nc.tensor.matmul(ps[0:100, 0:256],
                                             xt[ct][:, c0v:c0v+CT],
                                             wv[:, ct*256:ct*256+256],
                                             start=(ct == 0), stop=(ct == 1))
                        vch = v_pool.tile([100, 256], BF16, tag=f"vch{k}",
                                          name=f"vch_{b}_{i}_{k}")
                        with nc.allow_low_precision(reason="bf16 v"):
                            nc.vector.tensor_copy(vch[:], ps[0:100, 0:256])
                        vchs[4 * i + k] = vch

            def qk_tile(i):
                for j in range(4):
                    qk_pairs(i, j)

            avsb = av_pool.tile([128, 2 * BLK_T], BF16, tag="avsb",
                                name=f"avsb_{b}")
            outT = [o_pool.tile([128, BLK_T], BF16, tag=f"outT{t}",
                                name=f"outT{t}_{b}") for t in range(2)]
            prev = None        # (attn, vch, c0) of previous chunk

            def consume(attn, vch, c0, cl):
                # AV: 8 col-tiled matmuls -> dense psav [32*(h%4)+d, 100*(h//4)]
                # single bank: AV cols 0:200, denominators cols 256:456
                psav = ps_av.tile([128, 512], F32, tag="psav",
                                  name=f"psav_{b}_{cl}")
                for h in range(H):
                    s, j = h // 4, h % 4
                    nc.tensor.matmul(
                        psav[32*j:32*j+32, 100*s:100*s+100],
                        vch[:, 32*h:32*h+32], attn[:, 100*h:100*h+100],
                        start=True, stop=True, tile_position=(0, 32*j))
                for h in range(H):
                    s, j = h // 4, h % 4
                    nc.tensor.matmul(
                        psav[32*j:32*j+32, 256+100*s:256+100*s+100],
                        ones32[:], attn[:, 100*h:100*h+100],
                        start=True, stop=True, tile_position=(0, 32*j))
                rcpD = r_pool.tile([128, 200], F32, tag="rcp",
                                   name=f"rcp_{b}_{cl}")
                if SAFE_RCP:
                    nc.vector.reciprocal(rcpD[:], psav[:, 256:456])
                else:
                    nc.vector.reciprocal_approx_fast(rcpD[:], psav[:, 256:456])
                avR = avsb[:].rearrange("p (s n) -> p s n", s=2)
                with nc.allow_low_precision(reason="evac mult"):
                    nc.vector.tensor_mul(
                        avR[:, :, c0:c0+100],
                        psav[:, 0:200].rearrange("p (s n) -> p s n", s=2),
                        rcpD[:].rearrange("p (s n) -> p s n", s=2))

            def proj(nt):
                # reversed proj: outT[c,q] accumulated over 2 hd-subtiles
                for t in range(2):
                    pso = ps_o.tile([128, 400], F32, tag="pso",
                                    name=f"pso_{b}_{nt}_{t}")
                    for s in range(2):
                        nc.tensor.matmul(
                            pso[:], wpT[:, 256*s+128*t:256*s+128*t+128],
                            avsb[:, BLK_T*s + 400*nt: BLK_T*s + 400*nt + 400],
                            start=(s == 0), stop=(s == 1))
                    with nc.allow_low_precision(reason="bf16 out"):
                        nc.scalar.activation(
                            outT[t][:, 400*nt:400*nt+400], pso[:],
                            mybir.ActivationFunctionType.Copy)

            qk_tile(0)
            for cl in range(BLK_CH):
                c0 = cl * CT
                if cl // 4 + 1 < 8:
                    qk_pairs(cl // 4 + 1, cl % 4)
                if cl == 12 and b + 1 < NBLK:
                    xt_cur = load_xt(b + 1)
                vch = vchs.pop(cl)
                # pss: 4 banks, one per row strip: head (g,hl) at cols
                # 512*hl + 100*g
                pss = ps_s.tile([100, 2048], F32, tag="pss",
                                name=f"pss_{b}_{cl}")
                # row-tiled scores, each strip drains to its own bank
                for hl in (0, 1, 2, 3):
                    for g in range(2):
                        o = 512*hl + 100*g
                        nc.tensor.matmul(
                            pss[:, o:o+100],
                            qk[2+g][32*hl:32*hl+32, c0:c0+CT],
                            qk[g][32*hl:32*hl+32, c0:c0+CT],
                            start=True, stop=True,
                            tile_position=(32*hl, 0))
                attn_raw = a_pool.tile([100, 800], BF16, tag="attn_raw",
                                       name=f"attn_raw_{b}_{cl}")
                src = pss[:].rearrange("p (hl x) -> p hl x", hl=4)
                src = src[:, :, 0:200].rearrange("p hl (g n) -> p g hl n", g=2)
                dst = attn_raw[:].rearrange("p (g hl n) -> p g hl n",
                                            g=2, hl=4)
                with nc.allow_low_precision(reason="bf16 attn"):
                    nc.scalar.activation(
                        dst, src, mybir.ActivationFunctionType.Exp)
                # bias+mask: attn = exp(s)*exp(b); junk entries get *0
                attn = a_pool.tile([100, 800], BF16, tag="attn",
                                   name=f"attn_{b}_{cl}")
                with nc.allow_low_precision(reason="bf16 attn"):
                    nc.vector.tensor_mul(attn[:, 0:400], attn_raw[:, 0:400],
                                         expB[:, 0:400])
                    nc.gpsimd.tensor_mul(attn[:, 400:800],
                                         attn_raw[:, 400:800],
                                         expB[:, 400:800])

                if prev is not None:
                    consume(*prev, cl - 1)
                    if (cl - 1) % 4 == 3:
                        proj((cl - 1) // 4)
                prev = (attn, vch, c0)
            consume(*prev, BLK_CH - 1)
            proj((BLK_CH - 1) // 4)
            for t in range(2):
                nc.sync.dma_start(out=out_d[:, T*t + t0: T*t + t0 + BLK_T],
                                  in_=outT[t][:])
    nc.compile()
    _cache['nc'] = nc
    return nc


def _host_prep(x, qkv_w, qkv_b, proj_w, proj_b, bias_table, rel_idx):
    f = np.float32
    scale = f(HD) ** -0.5
    qkv_w = np.asarray(qkv_w, f)
    qkv_b = np.asarray(qkv_b, f)
    proj_w = np.asarray(proj_w, f)
    proj_b = np.asarray(proj_b, f)
    if np.any(qkv_b[512:]) or np.any(proj_b):
        raise NotImplementedError("nonzero v/proj bias not supported")
    wq = qkv_w[0:256] * scale
    wk = qkv_w[256:512]
    wvm = qkv_w[512:768]
    # qk weights: lhsT layout [K=256 (2 ct-tiles of 128), M=512]
    w_qkT = np.concatenate([wq, wk], axis=0).T          # [256, 512]
    wqk_h = w_qkT.reshape(2, 128, 512).transpose(1, 0, 2).reshape(128, 1024)
    # v weights as rhs [K=256 -> 2x128, 256]
    w_vT = wvm.T                                        # [256, 256]
    wv_h = w_vT.reshape(2, 128, 256).transpose(1, 0, 2).reshape(128, 512)
    # reversed proj weights: wpT[32j+d, 256s+128t+c'] = proj_w[128t+c', (4s+j)*32+d]
    wp_h = np.zeros((128, 512), f)
    for s in range(2):
        for t in range(2):
            blk = proj_w[128*t:128*t+128, 128*s:128*s+128]   # [c', (j,d)]
            wp_h[:, 256*s+128*t:256*s+128*t+128] = blk.T
    # q/k bias per-partition [128, 4] (mt = q g0, q g1, k g0, k g1)
    qb_eff = qkv_b.copy()
    qb_eff[0:256] *= scale
    qb_h = qb_eff[0:512].reshape(4, 128).T.copy()       # [128, 4]
    # scoresT exp-bias [key 100, (h, q) 800]: exp(bias) valid, 0 at junk
    biasH = np.asarray(bias_table, f)[np.asarray(rel_idx)]      # [49,49,H]
    biasH = np.pad(biasH, ((1, 0), (1, 0), (0, 0)))             # [50,50,H]
    biasH = biasH.transpose(2, 0, 1)                            # [H, q, key]
    bT = np.zeros((100, 8, 100), f)
    for w in range(2):
        blk = np.exp(biasH.transpose(0, 2, 1))                  # [H, key, q]
        bT[50*w:50*w+50, :, 50*w:50*w+50] = blk.transpose(1, 0, 2)
    bias_h = bT.reshape(100, 800)
    bf = ml_dtypes.bfloat16
    return (wqk_h.astype(bf), wv_h.astype(bf), wp_h.astype(bf),
            bias_h.astype(bf), qb_h)


def kernel(x, qkv_w, qkv_b, proj_w, proj_b, bias_table, rel_idx):
    wqk_h, wv_h, wp_h, bias_h, qb_h = _host_prep(
        x, qkv_w, qkv_b, proj_w, proj_b, bias_table, rel_idx)
    bf = ml_dtypes.bfloat16
    x_bf = np.ascontiguousarray(np.asarray(x, np.float32)).astype(bf)
    x_sh = x_bf.reshape(N_CORES, T, DIM)
    nc = _build_program()
    in_maps = [{"x": x_sh[i], "wqk": wqk_h, "wv": wv_h, "wpT": wp_h,
                "biasT": bias_h, "qb": qb_h} for i in range(N_CORES)]
    trace = bool(os.environ.get("BASS_KERNEL_TRACE"))
    if trace:
        _install_ntff_shim()
    res = run_bass_kernel_spmd(nc, in_maps, list(range(N_CORES)), trace=trace)
    if trace and res.exec_time_ns is not None:
        print(f"HW exec time: {res.exec_time_ns} ns")
    # out_dT [128, 2*T]: out[tok, 128t+c'] = arr[c', T*t + tok]
    outs = []
    for i in range(N_CORES):
        arr = np.asarray(res.results[i]["out"]).astype(np.float32)
        outs.append(arr.reshape(128, 2, T).transpose(2, 1, 0).reshape(T, DIM))
    out = np.stack(outs, axis=0)
    return out.reshape(B_, WN, DIM)


# revision 37
# speedup vs baseline: 1.1618x; 1.1618x over previous
"""Trainium2 Bass kernel for windowed multi-head self-attention (Swin/LSA style).

Shapes (hardcoded): x [2048, 50, 256], 8 heads, head_dim 32, window N=50
(49 patch tokens + 1 region token), relative-position bias on the 49x49 block.

Strategy: data-parallel over the 2048 windows across 8 NeuronCores (256
windows/core). Per core, tokens are processed in chunks of 2 windows
(100 tokens), software-pipelined by one chunk so the PE never waits on the
exp activation. All matmuls bf16 on the PE with fp32 PSUM accumulate.

Per chunk (scoresT layout [key, (h,q)], no transposes, no block-diag build):
  scores: 8 row-tiled matmuls (K=32 per head, tile_position row strips) that
    accumulate onto a bias+mask pre-injected by 2 identity matmuls
  exp via one ACT op over both PSUM banks during evac
  AV: 8 col-tiled matmuls (M=32, tile_position col strips) -> dense PSUM
    layout [row 32*(h%4)+d, bank h//4]
  denominators: 8 col-tiled ones-matmuls into spare cols of the same banks
    (each [32,100] block = colsums broadcast over 32 rows, matching layout)
  reciprocal_approx_fast on the denom cols, 2 evac-multiplies -> dense avsb
  proj reversed (outT = Wp^T.T @ avsb): 2 accumulating matmuls per 400-col
    tile, output written transposed to HBM; host does the final transpose.
"""
import os
import sys
import numpy as np
import ml_dtypes

sys.path.insert(0, '/opt/trn_rl_repo')

import concourse.bacc as bacc
import concourse.mybir as mybir
from concourse import tile
from concourse.bass_utils import run_bass_kernel_spmd

BF16 = mybir.dt.bfloat16
F32 = mybir.dt.float32

N_CORES = 8
DIM = 256
H = 8
HD = 32
WN = 50                      # tokens per window
B_ = 2048
BPC = B_ // N_CORES          # windows per core
T = BPC * WN                 # tokens per core = 12800
CT = 100                     # tokens per chunk (2 windows)
NCHUNK = T // CT             # 128 chunks per core
BLK_CH = 32                  # chunks per block
BLK_T = BLK_CH * CT          # 3200 tokens per block
NBLK = NCHUNK // BLK_CH      # 4 blocks

_cache = {}
SAFE_RCP = bool(os.environ.get("K_SAFE_RCP"))
SAFE_SC = True   # grouped bias-inject + skip_group_check hangs on HW
SAFE_EXP = bool(os.environ.get("K_SAFE_EXP"))


def _install_ntff_shim():
    """Register the axon NTFF profile hook (antenv stub lacks axon_hooks)."""
    import types
    if 'antenv.axon_hooks' in sys.modules:
        return
    try:
        import antenv
        from trn_agent_boot.trn_boot import _ntff_profile_via_ctypes
    except ImportError:
        return
    hooks = types.ModuleType("antenv.axon_hooks")
    holder = {}
    hooks.set_axon_ntff_profile_hook = lambda h: holder.__setitem__('h', h)
    hooks.get_axon_ntff_profile_hook = lambda: holder.get('h')
    antenv.axon_hooks = hooks
    sys.modules['antenv.axon_hooks'] = hooks
    hook = _ntff_profile_via_ctypes('/opt/axon/libaxon_pjrt.so')
    if hook is not None:
        hooks.set_axon_ntff_profile_hook(hook)


def _build_program():
    if 'nc' in _cache:
        return _cache['nc']
    nc = bacc.Bacc("TRN2", target_bir_lowering=False, debug=False,
                   num_devices=N_CORES)
    x_d = nc.dram_tensor("x", [T, DIM], BF16, kind="ExternalInput").ap()
    wqk_d = nc.dram_tensor("wqk", [128, 1024], BF16, kind="ExternalInput").ap()
    wv_d = nc.dram_tensor("wv", [128, 512], BF16, kind="ExternalInput").ap()
    wp_d = nc.dram_tensor("wpT", [128, 512], BF16, kind="ExternalInput").ap()
    bias_d = nc.dram_tensor("biasT", [100, 800], BF16, kind="ExternalInput").ap()
    qb_d = nc.dram_tensor("qb", [128, 4], F32, kind="ExternalInput").ap()
    out_d = nc.dram_tensor("out", [128, 2 * T], BF16, kind="ExternalOutput").ap()

    from contextlib import ExitStack
    with tile.TileContext(nc) as tc, ExitStack() as es:
        cpool = es.enter_context(tc.tile_pool(name="consts", bufs=1))
        wqk = cpool.tile([128, 1024], BF16)       # [ct, 4mt x 128] qk weights
        nc.sync.dma_start(out=wqk[:], in_=wqk_d[:])
        wv = cpool.tile([128, 512], BF16)         # [ct, 256] v weights (rhs)
        nc.sync.dma_start(out=wv[:], in_=wv_d[:])
        wpT = cpool.tile([128, 512], BF16)        # [(j,d), (s,t,128)] proj^T
        nc.sync.dma_start(out=wpT[:], in_=wp_d[:])
        expB = cpool.tile([100, 800], BF16)       # exp(bias), 0 at junk
        nc.sync.dma_start(out=expB[:], in_=bias_d[:])
        qb = cpool.tile([128, 4], F32)            # q/k bias per-partition
        nc.sync.dma_start(out=qb[:], in_=qb_d[:])
        ones32 = cpool.tile([100, 32], BF16)      # denominator stationary
        nc.vector.memset(ones32[:], 1.0)

        xt_pool = es.enter_context(tc.tile_pool(name="xt", bufs=2))
        qk_pool = es.enter_context(tc.tile_pool(name="qk", bufs=2))
        v_pool = es.enter_context(tc.tile_pool(name="v", bufs=2))
        a_pool = es.enter_context(tc.tile_pool(name="attn", bufs=2))
        r_pool = es.enter_context(tc.tile_pool(name="rcp", bufs=2))
        av_pool = es.enter_context(tc.tile_pool(name="avsb", bufs=2))
        o_pool = es.enter_context(tc.tile_pool(name="osb", bufs=2))
        ps_qk = es.enter_context(tc.tile_pool(name="ps_qk", bufs=2, space="PSUM"))
        ps_s = es.enter_context(tc.tile_pool(name="ps_s", bufs=1, space="PSUM"))
        ps_av = es.enter_context(tc.tile_pool(name="ps_av", bufs=1, space="PSUM"))
        ps_o = es.enter_context(tc.tile_pool(name="ps_o", bufs=1, space="PSUM"))

        xt = [None, None]   # double-buffered via pool tags

        def load_xt(b):
            t0 = b * BLK_T
            tls = [xt_pool.tile([128, BLK_T], BF16, tag=f"xt{ct}",
                                name=f"xt{ct}_{b}") for ct in range(2)]
            for ct in range(2):
                nc.sync.dma_start(out=tls[ct][:],
                                  in_=x_d[t0:t0 + BLK_T, 128*ct:128*ct+128],
                                  transpose=True)
            return tls

        xt_cur = load_xt(0)
        for b in range(NBLK):
            t0 = b * BLK_T
            xt = xt_cur
            # qT/kT: 4 m-tiles [128, 3200] (q g0, q g1, k g0, k g1),
            # produced in 400-col tiles interleaved with chunk work below
            qk = [qk_pool.tile([128, BLK_T], BF16, tag=f"qk{mt}",
                               name=f"qk{mt}_{b}") for mt in range(4)]

            vchs = {}

            def qk_pairs(i, j):
                # j=0/1: qk m-tile pairs; j=2/3: v pairs — spread across the
                # group so the PE has filler work during every exp window
                ns = 400 * i
                if j < 2:
                    for mt in (2*j, 2*j+1):
                        ps = ps_qk.tile([128, 512], F32, tag="ps_qk")
                        for ct in range(2):
                            nc.tensor.matmul(
                                ps[:, 0:400],
                                wqk[:, ct*512 + mt*128: ct*512 + mt*128+128],
                                xt[ct][:, ns:ns+400],
                                start=(ct == 0), stop=(ct == 1))
                        with nc.allow_low_precision(reason="bf16 qk"):
                            if mt >= 2:
                                nc.vector.tensor_scalar_add(
                                    qk[mt][:, ns:ns+400], ps[:, 0:400],
                                    qb[:, mt:mt+1])
                            else:
                                nc.scalar.activation(
                                    qk[mt][:, ns:ns+400], ps[:, 0:400],
                                    mybir.ActivationFunctionType.Identity,
                                    bias=qb[:, mt:mt+1])
                else:
                    for k in (2*(j-2), 2*(j-2)+1):
                        c0v = ns + 100 * k
                        ps = ps_qk.tile([128, 512], F32, tag="ps_qk")
                        for ct in range(2):
                            nc.tensor.matmul(ps[0:100, 0:256],
                                             xt[ct][:, c0v:c0v+CT],
                                             wv[:, ct*256:ct*256+256],
                                             start=(ct == 0), stop=(ct == 1))
                        vch = v_pool.tile([100, 256], BF16, tag=f"vch{k}",
                                          name=f"vch_{b}_{i}_{k}")
                        with nc.allow_low_precision(reason="bf16 v"):
                            nc.vector.tensor_copy(vch[:], ps[0:100, 0:256])
                        vchs[4 * i + k] = vch

            def qk_tile(i):
                for j in range(4):
                    qk_pairs(i, j)

            avsb = av_pool.tile([128, 2 * BLK_T], BF16, tag="avsb",
                                name=f"avsb_{b}")
            outT = [o_pool.tile([128, BLK_T], BF16, tag=f"outT{t}",
                                name=f"outT{t}_{b}") for t in range(2)]
            prev = None        # (attn, vch, c0) of previous chunk

            def consume(attn, vch, c0, cl):
                # AV: 8 col-tiled matmuls -> dense psav [32*(h%4)+d, 100*(h//4)]
                # single bank: AV cols 0:200, denominators cols 256:456
                psav = ps_av.tile([128, 512], F32, tag="psav",
                                  name=f"psav_{b}_{cl}")
                for h in range(H):
                    s, j = h // 4, h % 4
                    nc.tensor.matmul(
                        psav[32*j:32*j+32, 100*s:100*s+100],
                        vch[:, 32*h:32*h+32], attn[:, 100*h:100*h+100],
                        start=True, stop=True, tile_position=(0, 32*j))
                for h in range(H):
                    s, j = h // 4, h % 4
                    nc.tensor.matmul(
                        psav[32*j:32*j+32, 256+100*s:256+100*s+100],
                        ones32[:], attn[:, 100*h:100*h+100],
                        start=True, stop=True, tile_position=(0, 32*j))
                rcpD = r_pool.tile([128, 200], F32, tag="rcp",
                                   name=f"rcp_{b}_{cl}")
                if SAFE_RCP:
                    nc.vector.reciprocal(rcpD[:], psav[:, 256:456])
                else:
                    nc.vector.reciprocal_approx_fast(rcpD[:], psav[:, 256:456])
                avR = avsb[:].rearrange("p (s n) -> p s n", s=2)
                with nc.allow_low_precision(reason="evac mult"):
                    nc.vector.tensor_mul(
                        avR[:, :, c0:c0+100],
                        psav[:, 0:200].rearrange("p (s n) -> p s n", s=2),
                        rcpD[:].rearrange("p (s n) -> p s n", s=2))

            def proj(nt):
                # reversed proj: outT[c,q] accumulated over 2 hd-subtiles
                for t in range(2):
                    pso = ps_o.tile([128, 400], F32, tag="pso",
                                    name=f"pso_{b}_{nt}_{t}")
                    for s in range(2):
                        nc.tensor.matmul(
                            pso[:], wpT[:, 256*s+128*t:256*s+128*t+128],
                            avsb[:, BLK_T*s + 400*nt: BLK_T*s + 400*nt + 400],
                            start=(s == 0), stop=(s == 1))
                    with nc.allow_low_precision(reason="bf16 out"):
                        nc.scalar.activation(
                            outT[t][:, 400*nt:400*nt+400], pso[:],
                            mybir.ActivationFunctionType.Copy)

            qk_tile(0)
            for cl in range(BLK_CH):
                c0 = cl * CT
                if cl == 12 and b + 1 < NBLK:
                    xt_cur = load_xt(b + 1)
                vch = vchs.pop(cl)
                # pss: 4 banks, one per row strip: head (g,hl) at cols
                # 512*hl + 100*g
                pss = ps_s.tile([100, 2048], F32, tag="pss",
                                name=f"pss_{b}_{cl}")
                # row-tiled scores, each strip drains to its own bank
                for hl in (0, 1, 2, 3):
                    for g in range(2):
                        o = 512*hl + 100*g
                        nc.tensor.matmul(
                            pss[:, o:o+100],
                            qk[2+g][32*hl:32*hl+32, c0:c0+CT],
                            qk[g][32*hl:32*hl+32, c0:c0+CT],
                            start=True, stop=True,
                            tile_position=(32*hl, 0))
                attn_raw = a_pool.tile([100, 800], BF16, tag="attn_raw",
                                       name=f"attn_raw_{b}_{cl}")
                src = pss[:].rearrange("p (hl x) -> p hl x", hl=4)
                src = src[:, :, 0:200].rearrange("p hl (g n) -> p g hl n", g=2)
                dst = attn_raw[:].rearrange("p (g hl n) -> p g hl n",
                                            g=2, hl=4)
                with nc.allow_low_precision(reason="bf16 attn"):
                    nc.scalar.activation(
                        dst, src, mybir.ActivationFunctionType.Exp)
                # bias+mask: attn = exp(s)*exp(b); junk entries get *0
                attn = a_pool.tile([100, 800], BF16, tag="attn",
                                   name=f"attn_{b}_{cl}")
                with nc.allow_low_precision(reason="bf16 attn"):
                    nc.gpsimd.tensor_mul(attn[:], attn_raw[:], expB[:])

                # filler for the exp-wait window: next tile's qk/v pairs
                if cl // 4 + 1 < 8:
                    qk_pairs(cl // 4 + 1, cl % 4)

                if prev is not None:
                    consume(*prev, cl - 1)
                    if (cl - 1) % 4 == 3:
                        proj((cl - 1) // 4)
                prev = (attn, vch, c0)
            consume(*prev, BLK_CH - 1)
            proj((BLK_CH - 1) // 4)
            for t in range(2):
                nc.sync.dma_start(out=out_d[:, T*t + t0: T*t + t0 + BLK_T],
                                  in_=outT[t][:])
    nc.compile()
    _cache['nc'] = nc
    return nc


def _host_prep(x, qkv_w, qkv_b, proj_w, proj_b, bias_table, rel_idx):
    f = np.float32
    scale = f(HD) ** -0.5
    qkv_w = np.asarray(qkv_w, f)
    qkv_b = np.asarray(qkv_b, f)
    proj_w = np.asarray(proj_w, f)
    proj_b = np.asarray(proj_b, f)
    if np.any(qkv_b[512:]) or np.any(proj_b):
        raise NotImplementedError("nonzero v/proj bias not supported")
    wq = qkv_w[0:256] * scale
    wk = qkv_w[256:512]
    wvm = qkv_w[512:768]
    # qk weights: lhsT layout [K=256 (2 ct-tiles of 128), M=512]
    w_qkT = np.concatenate([wq, wk], axis=0).T          # [256, 512]
    wqk_h = w_qkT.reshape(2, 128, 512).transpose(1, 0, 2).reshape(128, 1024)
    # v weights as rhs [K=256 -> 2x128, 256]
    w_vT = wvm.T                                        # [256, 256]
    wv_h = w_vT.reshape(2, 128, 256).transpose(1, 0, 2).reshape(128, 512)
    # reversed proj weights: wpT[32j+d, 256s+128t+c'] = proj_w[128t+c', (4s+j)*32+d]
    wp_h = np.zeros((128, 512), f)
    for s in range(2):
        for t in range(2):
            blk = proj_w[128*t:128*t+128, 128*s:128*s+128]   # [c', (j,d)]
            wp_h[:, 256*s+128*t:256*s+128*t+128] = blk.T
    # q/k bias per-partition [128, 4] (mt = q g0, q g1, k g0, k g1)
    qb_eff = qkv_b.copy()
    qb_eff[0:256] *= scale
    qb_h = qb_eff[0:512].reshape(4, 128).T.copy()       # [128, 4]
    # scoresT exp-bias [key 100, (h, q) 800]: exp(bias) valid, 0 at junk
    biasH = np.asarray(bias_table, f)[np.asarray(rel_idx)]      # [49,49,H]
    biasH = np.pad(biasH, ((1, 0), (1, 0), (0, 0)))             # [50,50,H]
    biasH = biasH.transpose(2, 0, 1)                            # [H, q, key]
    bT = np.zeros((100, 8, 100), f)
    for w in range(2):
        blk = np.exp(biasH.transpose(0, 2, 1))                  # [H, key, q]
        bT[50*w:50*w+50, :, 50*w:50*w+50] = blk.transpose(1, 0, 2)
    bias_h = bT.reshape(100, 800)
    bf = ml_dtypes.bfloat16
    return (wqk_h.astype(bf), wv_h.astype(bf), wp_h.astype(bf),
            bias_h.astype(bf), qb_h)


def kernel(x, qkv_w, qkv_b, proj_w, proj_b, bias_table, rel_idx):
    wqk_h, wv_h, wp_h, bias_h, qb_h = _host_prep(
        x, qkv_w, qkv_b, proj_w, proj_b, bias_table, rel_idx)
    bf = ml_dtypes.bfloat16
    x_bf = np.ascontiguousarray(np.asarray(x, np.float32)).astype(bf)
    x_sh = x_bf.reshape(N_CORES, T, DIM)
    nc = _build_program()
    in_maps = [{"x": x_sh[i], "wqk": wqk_h, "wv": wv_h, "wpT": wp_h,
                "biasT": bias_h, "qb": qb_h} for i in range(N_CORES)]
    trace = bool(os.environ.get("BASS_KERNEL_TRACE"))
    if trace:
        _install_ntff_shim()
    res = run_bass_kernel_spmd(nc, in_maps, list(range(N_CORES)), trace=trace)
    if trace and res.exec_time_ns is not None:
        print(f"HW exec time: {res.exec_time_ns} ns")
    # out_dT [128, 2*T]: out[tok, 128t+c'] = arr[c', T*t + tok]
    outs = []
    for i in range(N_CORES):
        arr = np.asarray(res.results[i]["out"]).astype(np.float32)
        outs.append(arr.reshape(128, 2, T).transpose(2, 1, 0).reshape(T, DIM))
    out = np.stack(outs, axis=0)
    return out.reshape(B_, WN, DIM)
